# revision 5
# baseline (speedup 1.0000x reference)
"""Causal self-attention (B=1, T=4096, C=768, H=12, hd=64) on 8 trn2 NeuronCores.

Strategy (all FLOPs on device, host only reshapes/slices):
  Launch 1 (sequence-parallel): core c computes qkv for rows [512c, 512c+512):
    q^T, k^T produced directly in [channel, pos] layout via W^T @ x^T (bf16).
    RoPE: rope(a) = a * cos + rot(a) * sin, where rot(a) = P @ a is a signed
    128-partition permutation applied with a single PE matmul per tile
    (P^T stationary), so no rotated copy of W is needed.
    v produced in natural [pos, channel] layout.  All outputs bf16.
  Launch 2 (query-block-parallel): core c owns 4 query blocks of 128 rows
    [31-c, 16+c, 15-c, c] (sorted by descending causal length; padded kv-tile
    counts 32/24/16/8 -- identical SPMD program on every core).  Scores are
    computed transposed S^T[kv, q] in bf16; causal/padding masks are rank-4
    augmentations of the contraction (4 extra "mask channels" in q^T/k^T);
    the per-block diagonal kv tile is processed separately with a constant
    triangular additive mask.  The kv-tile schedule uses the minimal uniform
    width profile 512/384/256/128 (x8 tiles each).  exp via ScalarE (no
    row-max needed: scores are N(0,1)-scale), denominator via an appended
    ones-column on V, per-head normalization via a PE-broadcast reciprocal,
    then the output projection contracts y^T directly (no transpose), bias
    b_proj' = b_proj + bv @ w_proj folded on host.
"""

import numpy as np
import ml_dtypes

import concourse.bass as bass
import concourse.bacc as bacc
import concourse.tile as tile
from concourse import mybir
from concourse.bass_utils import run_bass_kernel_spmd

F32 = mybir.dt.float32
F32R = mybir.dt.float32r
BF16 = mybir.dt.bfloat16
NPBF16 = ml_dtypes.bfloat16

T, C, H, HD = 4096, 768, 12, 64
NCORES = 8
RPC = T // NCORES          # rows per core in launch 1 (512)
NT = T // 128              # kv tiles (32)
MASK = -2000.0             # additive mask; *0.125 -> exp underflows to 0
ROPE_BASE = 10000.0

# launch-2 slot structure: slot s of core c handles query block BLOCKS[c][s]
# (this order is sorted by descending causal length for every c)
BLOCKS = [[31 - c, 16 + c, 15 - c, c] for c in range(NCORES)]
# padded kv-tile counts per slot position (max over cores of true counts)
PAD = [32, 24, 16, 8]

# main-loop schedule: groups of (exp span, [(kv tile, psum col off, width)]).
# width(t) = 128 * #slot-positions whose padded count-1 exceeds t; the
# diagonal tile of each slot is handled separately (ktd/vd + tri mask).
SCHED = [
    (1024, [(0, 0, 512), (1, 512, 512)]),
    (1024, [(2, 0, 512), (3, 512, 512)]),
    (1024, [(4, 0, 512), (5, 512, 512)]),
    (896,  [(6, 0, 512), (7, 512, 384)]),
    (1024, [(8, 0, 384), (23, 384, 128), (9, 512, 384), (24, 896, 128)]),
    (1024, [(10, 0, 384), (25, 384, 128), (11, 512, 384), (26, 896, 128)]),
    (1024, [(12, 0, 384), (27, 384, 128), (13, 512, 384), (28, 896, 128)]),
    (1024, [(14, 0, 384), (29, 384, 128), (15, 512, 256), (16, 768, 256)]),
    (1024, [(17, 0, 256), (18, 256, 256), (19, 512, 256), (20, 768, 256)]),
    (640,  [(21, 0, 256), (22, 256, 256), (30, 512, 128)]),
]


def _build_l1(reps=1):
    nc = bacc.Bacc("TRN2", target_bir_lowering=False, debug=False,
                   num_devices=NCORES)
    XT = nc.dram_tensor("xt", [128, 6, RPC], BF16, kind="ExternalInput")
    WA = nc.dram_tensor("wa", [128, 6, 3 * C], BF16, kind="ExternalInput")
    PT = nc.dram_tensor("pt", [128, 128], BF16, kind="ExternalInput")
    BQK = nc.dram_tensor("bqk", [128, 12], F32, kind="ExternalInput")
    COS = nc.dram_tensor("cos", [128, RPC], BF16, kind="ExternalInput")
    SIN = nc.dram_tensor("sin", [128, RPC], F32, kind="ExternalInput")
    QKT = nc.dram_tensor("qkt", [2 * C, RPC], BF16, kind="ExternalOutput")
    VO = nc.dram_tensor("vo", [RPC, C], BF16, kind="ExternalOutput")

    with tile.TileContext(nc) as tc:
        with (
            tc.tile_pool(name="singles", bufs=1) as singles,
            tc.tile_pool(name="tmp", bufs=3) as tmp,
            tc.tile_pool(name="ps", bufs=2, space="PSUM") as ps,
        ):
            # weight tiles for q,k: wa cols [0, 1536); v weights: [1536, 2304)
            xt_sb = singles.tile([128, 6, RPC], BF16)
            nc.sync.dma_start(out=xt_sb, in_=XT[:])
            wqk_sb = singles.tile([128, 6, 2 * C], BF16)
            nc.sync.dma_start(out=wqk_sb[:, :, 0:512], in_=WA[:, :, 0:512])
            nc.scalar.dma_start(out=wqk_sb[:, :, 512:1024],
                                in_=WA[:, :, 512:1024])
            nc.gpsimd.dma_start(out=wqk_sb[:, :, 1024:1536],
                                in_=WA[:, :, 1024:1536])
            wv_sb = singles.tile([128, 6, C], BF16)
            nc.gpsimd.dma_start(out=wv_sb, in_=WA[:, :, 1536:2304])
            pt_sb = singles.tile([128, 128], BF16)
            nc.gpsimd.dma_start(out=pt_sb, in_=PT[:])
            bqk_sb = singles.tile([128, 12], F32)
            nc.gpsimd.dma_start(out=bqk_sb, in_=BQK[:])
            cos_sb = singles.tile([128, RPC], BF16)
            nc.scalar.dma_start(out=cos_sb, in_=COS[:])
            sin_sb = singles.tile([128, RPC], F32)
            nc.scalar.dma_start(out=sin_sb, in_=SIN[:])

            qdma = [nc.sync, nc.scalar, nc.gpsimd]

            def body(_=None):
                # q^T, k^T with RoPE: 12 channel tiles of 128
                for m in range(12):
                    ps_a = ps.tile([128, RPC], F32, tag="psa")
                    for k in range(6):
                        nc.tensor.matmul(
                            ps_a, wqk_sb[:, k, 128 * m:128 * (m + 1)],
                            xt_sb[:, k, :], start=(k == 0), stop=(k == 5))
                    a_sb = tmp.tile([128, RPC], BF16, tag="a")
                    nc.scalar.activation(a_sb, ps_a,
                                         mybir.ActivationFunctionType.Identity,
                                         bias=bqk_sb[:, m:m + 1])
                    ps_b = ps.tile([128, RPC], F32, tag="psb")
                    nc.tensor.matmul(ps_b, pt_sb, a_sb, start=True, stop=True)
                    t1 = tmp.tile([128, RPC], BF16, tag="t1")
                    nc.vector.tensor_mul(t1, a_sb, cos_sb)
                    t2 = tmp.tile([128, RPC], F32, tag="t2")
                    nc.vector.tensor_mul(t2, ps_b, sin_sb)
                    o_sb = tmp.tile([128, RPC], BF16, tag="o")
                    nc.vector.tensor_add(o_sb, t1, t2)
                    qdma[m % 3].dma_start(
                        out=QKT[128 * m:128 * (m + 1), :], in_=o_sb)

                # v in natural layout: 4 row tiles x (512 + 256) cols
                for qt in range(4):
                    ps_v = ps.tile([128, C], F32, tag="psv")
                    for n0, nw in ((0, 512), (512, 256)):
                        for k in range(6):
                            nc.tensor.matmul(
                                ps_v[:, n0:n0 + nw],
                                xt_sb[:, k, 128 * qt:128 * (qt + 1)],
                                wv_sb[:, k, n0:n0 + nw],
                                start=(k == 0), stop=(k == 5))
                    vo_sb = tmp.tile([128, C], BF16, tag="vo")
                    nc.scalar.activation(vo_sb, ps_v,
                                         mybir.ActivationFunctionType.Identity)
                    nc.gpsimd.dma_start(
                        out=VO[128 * qt:128 * (qt + 1), :], in_=vo_sb)

            if reps == 1:
                body()
            else:
                with tc.For_i(0, reps, 1):
                    body()
    nc.finalize()
    return nc


def _build_l2(reps=1):
    nc = bacc.Bacc("TRN2", target_bir_lowering=False, debug=False,
                   num_devices=NCORES)
    KTM = nc.dram_tensor("ktm", [H, 68, T], BF16, kind="ExternalInput")
    QTM = nc.dram_tensor("qtm", [H, 68, 512], BF16, kind="ExternalInput")
    VP = nc.dram_tensor("vp", [H, 128, NT * (HD + 1)], BF16, kind="ExternalInput")
    KTD = nc.dram_tensor("ktd", [H, 68, 512], BF16, kind="ExternalInput")
    VD = nc.dram_tensor("vd", [H, 128, 4 * (HD + 1)], BF16, kind="ExternalInput")
    TRI = nc.dram_tensor("tri", [128, 512], F32, kind="ExternalInput")
    WP = nc.dram_tensor("wp", [C, C], BF16, kind="ExternalInput")
    ONESR = nc.dram_tensor("onesr", [1, 64], F32R, kind="ExternalInput")
    BP = nc.dram_tensor("bp", [1, C], F32, kind="ExternalInput")
    OUT = nc.dram_tensor("out", [512, C], F32, kind="ExternalOutput")

    with tile.TileContext(nc) as tc:
        with (
            tc.tile_pool(name="singles", bufs=1) as singles,
            tc.tile_pool(name="big", bufs=3) as big,
            tc.tile_pool(name="pt", bufs=5) as ptp,
            tc.tile_pool(name="small", bufs=3) as small,
            tc.tile_pool(name="sp", bufs=2, space="PSUM") as sp,
            tc.tile_pool(name="spd", bufs=1, space="PSUM") as spd,
            tc.tile_pool(name="yp", bufs=2, space="PSUM") as yp,
            tc.tile_pool(name="rp", bufs=1, space="PSUM") as rp,
        ):
            # one-time loads on the SWDGE queue (Pool) so they don't block
            # the per-head loads; ACT's queue is kept free for exp.
            wp_sb = singles.tile([128, 6, C], BF16)
            tri_sb = singles.tile([128, 512], F32)
            nc.gpsimd.dma_start(out=tri_sb, in_=TRI[:])
            bp_sb = singles.tile([128, C], F32)
            ones65 = singles.tile([65, 64], F32R)
            nc.gpsimd.dma_start(out=ones65[64:65, :], in_=ONESR[:])
            yt_sb = singles.tile([128, 6, 512], BF16)

            def load_head(h):
                kth = big.tile([68, T], BF16, tag="kth")
                nc.sync.dma_start(out=kth, in_=KTM[h])
                vh = big.tile([128, NT, HD + 1], BF16, tag="vh")
                nc.gpsimd.dma_start(out=vh, in_=VP[h])
                qth = small.tile([68, 512], BF16, tag="qth")
                nc.sync.dma_start(out=qth, in_=QTM[h])
                ktd = small.tile([68, 512], BF16, tag="ktd")
                nc.gpsimd.dma_start(out=ktd, in_=KTD[h])
                vd_sb = small.tile([128, 4, HD + 1], BF16, tag="vd")
                nc.gpsimd.dma_start(out=vd_sb, in_=VD[h])
                return kth, qth, vh, ktd, vd_sb

            def compute_head(h, tiles):
                kth, qth, vh, ktd, vd_sb = tiles
                y_ps = yp.tile([65, 512], F32, tag="y")

                # diag tiles: QK+mask emitted early (gap filler), exp late
                s2d = spd.tile([128, 512], F32, tag="s2d")
                ptd = ptp.tile([128, 512], BF16, tag="ptd")

                def emit_diag_front():
                    for s in range(4):
                        nc.tensor.matmul(
                            s2d[:, 128 * s:128 * (s + 1)],
                            ktd[:, 128 * s:128 * (s + 1)],
                            qth[:, 128 * s:128 * (s + 1)],
                            start=True, stop=True)
                    nc.vector.tensor_add(s2d, s2d, tri_sb)

                def emit_diag_exp():
                    nc.scalar.activation(ptd, s2d,
                                         mybir.ActivationFunctionType.Exp,
                                         scale=0.125)

                pending = None       # (items, pt2) awaiting AV
                for gi, (span, items) in enumerate(SCHED):
                    s2 = sp.tile([128, 1024], F32, tag="s2")
                    pt2 = ptp.tile([128, 1024], BF16, tag="pt2")
                    for (t, off, w) in items:
                        nc.tensor.matmul(
                            s2[:, off:off + w],
                            kth[:, 128 * t:128 * (t + 1)],
                            qth[:, 0:w],
                            start=True, stop=True)
                    if pending is not None:
                        pitems, ppt = pending
                        for (t, off, w) in pitems:
                            nc.tensor.matmul(
                                y_ps[:, 0:w], vh[:, t, :], ppt[:, off:off + w],
                                start=(t == 0), stop=False,
                                skip_group_check=True)
                    nc.scalar.activation(pt2[:, 0:span], s2[:, 0:span],
                                         mybir.ActivationFunctionType.Exp,
                                         scale=0.125)
                    if gi == 0:
                        emit_diag_front()
                    if gi == 8:
                        emit_diag_exp()
                    pending = (items, pt2)
                pitems, ppt = pending
                for (t, off, w) in pitems:
                    nc.tensor.matmul(
                        y_ps[:, 0:w], vh[:, t, :], ppt[:, off:off + w],
                        start=False, stop=False, skip_group_check=True)
                for s in range(4):
                    nc.tensor.matmul(
                        y_ps[:, 128 * s:128 * (s + 1)],
                        vd_sb[:, s, :], ptd[:, 128 * s:128 * (s + 1)],
                        start=False, stop=(s == 3), skip_group_check=True)

                # per-head normalization: yt[:, h, :] = y / sums
                rec = small.tile([65, 512], F32R, tag="rec")
                with nc.allow_low_precision(reason="f32r is fp32-width"):
                    nc.vector.reciprocal(rec[64:65, :], y_ps[64:65, :])
                rb_ps = rp.tile([64, 512], F32, tag="rb")
                nc.tensor.matmul(rb_ps, ones65[64:65, :], rec[64:65, :],
                                 start=True, stop=True)
                rb_sb = small.tile([64, 512], F32, tag="rbs")
                nc.vector.tensor_copy(rb_sb, rb_ps)
                if h % 2 == 0:
                    nc.vector.tensor_mul(yt_sb[0:64, h // 2, :],
                                         y_ps[0:64, :], rb_sb)
                else:
                    ytmp = small.tile([64, 512], BF16, tag="ytmp")
                    nc.vector.tensor_mul(ytmp, y_ps[0:64, :], rb_sb)
                    nc.sync.dma_start(out=yt_sb[64:128, h // 2, :], in_=ytmp)

            def body(_=None):
                cur = load_head(0)
                nc.gpsimd.dma_start(
                    out=wp_sb, in_=WP.rearrange("(k p) n -> p k n", p=128))
                nc.gpsimd.dma_start(out=bp_sb, in_=bass.AP(
                    tensor=BP, offset=0, ap=[[0, 128], [1, C]]))
                for h in range(H):
                    nxt = load_head(h + 1) if h + 1 < H else None
                    compute_head(h, cur)
                    cur = nxt
                # output projection: OUT[q, :] = y^T.T @ WP + BP
                for qt in range(4):
                    po = sp.tile([128, 1024], F32, tag="s2")
                    for n0, nw in ((0, 512), (512, 256)):
                        for k in range(6):
                            nc.tensor.matmul(
                                po[:, n0:n0 + nw],
                                yt_sb[:, k, 128 * qt:128 * (qt + 1)],
                                wp_sb[:, k, n0:n0 + nw],
                                start=(k == 0), stop=(k == 5))
                    ob = small.tile([128, C], F32, tag="ob")
                    nc.vector.tensor_add(ob, po[:, 0:C], bp_sb)
                    nc.sync.dma_start(out=OUT[128 * qt:128 * (qt + 1), :], in_=ob)

            if reps == 1:
                body()
            else:
                with tc.For_i(0, reps, 1):
                    body()
    nc.finalize()
    return nc


_CACHE = {}


def _get(name, builder):
    if name not in _CACHE:
        _CACHE[name] = builder()
    return _CACHE[name]


def _rot_pt():
    """P^T for rot(a) = P @ a: per-64-channel block, out[0:32] = -in[32:64],
    out[32:64] = in[0:32]."""
    p = np.zeros((128, 128), np.float32)
    for b in (0, 64):
        for i in range(32):
            p[b + i, b + 32 + i] = -1.0
            p[b + 32 + i, b + i] = 1.0
    return np.ascontiguousarray(p.T).astype(NPBF16)


def _prep_l1_inputs(x, w_attn, b_attn):
    xT = np.ascontiguousarray(x[0].T)                       # [C, T]
    wa3 = np.ascontiguousarray(
        w_attn.reshape(6, 128, 3 * C).transpose(1, 0, 2)).astype(NPBF16)
    bqk = np.ascontiguousarray(b_attn[:2 * C].reshape(12, 128).T)
    pt = _rot_pt()
    inv_freq = (1.0 / ROPE_BASE ** (np.arange(0, HD, 2, dtype=np.float64) / HD))
    d_idx = np.arange(128) % (HD // 2)
    in_maps = []
    for c in range(NCORES):
        t_rng = np.arange(RPC * c, RPC * (c + 1), dtype=np.float64)
        ang = np.outer(inv_freq[d_idx], t_rng)              # [128, RPC]
        xt3 = np.ascontiguousarray(
            xT[:, RPC * c:RPC * (c + 1)].reshape(6, 128, RPC)
            .transpose(1, 0, 2)).astype(NPBF16)
        in_maps.append({
            "xt": xt3, "wa": wa3, "pt": pt, "bqk": bqk,
            "cos": np.cos(ang).astype(NPBF16),
            "sin": np.sin(ang).astype(np.float32),
        })
    return in_maps


def _perm_v(v3):
    """[T', H, HD+1] -> [H, 128, (T'/128)*(HD+1)] partition-major."""
    tt = v3.shape[0]
    # [t, p, h, c] -> [h, p, t, c]
    v4 = v3.reshape(tt // 128, 128, H, HD + 1).transpose(2, 1, 0, 3)
    return np.ascontiguousarray(v4.reshape(H, 128, (tt // 128) * (HD + 1)))


def _prep_l2_inputs(QT_all, KT_all, Vp, w_proj, bp1):
    QT_all = np.asarray(QT_all, NPBF16)
    KT_all = np.asarray(KT_all, NPBF16)
    Vp = np.asarray(Vp, NPBF16)
    qm = np.zeros((4, 512), NPBF16)
    for s in range(4):
        qm[s, 128 * s:128 * (s + 1)] = 1.0
    tri = np.where(np.arange(128)[None, :] >= np.arange(128)[:, None],
                   0.0, MASK).astype(np.float32)
    tri4 = np.ascontiguousarray(np.tile(tri, (1, 4)))       # [128, 512]
    Vpp = _perm_v(Vp)
    in_maps = []
    for c in range(NCORES):
        blocks = BLOCKS[c]
        counts = [b + 1 for b in blocks]
        qt_c = np.concatenate(
            [QT_all[:, 128 * b:128 * (b + 1)] for b in blocks], axis=1)
        km = np.zeros((4, T), NPBF16)
        for s in range(4):
            km[s, 128 * (counts[s] - 1):] = MASK
        ktd_c = np.concatenate(
            [KT_all[:, 128 * b:128 * (b + 1)] for b in blocks], axis=1)
        vd = _perm_v(np.concatenate(
            [Vp[128 * b:128 * (b + 1)] for b in blocks], axis=0))
        # per-head packs with mask channels as rows 64:68
        ktm = np.zeros((H, 68, T), NPBF16)
        qtm = np.zeros((H, 68, 512), NPBF16)
        ktd3 = np.zeros((H, 68, 512), NPBF16)
        for h in range(H):
            ktm[h, 0:64] = KT_all[64 * h:64 * (h + 1)]
            ktm[h, 64:68] = km
            qtm[h, 0:64] = qt_c[64 * h:64 * (h + 1)]
            qtm[h, 64:68] = qm
            ktd3[h, 0:64] = ktd_c[64 * h:64 * (h + 1)]
        in_maps.append({
            "ktm": ktm, "qtm": qtm, "vp": Vpp, "ktd": ktd3,
            "vd": vd, "tri": tri4,
            "wp": np.asarray(w_proj, NPBF16), "bp": bp1.reshape(1, C),
            "onesr": np.ones((1, 64), np.float32),
        })
    return in_maps


def kernel(x, w_attn, b_attn, w_proj, b_proj):
    x = np.asarray(x, np.float32)
    w_attn = np.asarray(w_attn, np.float32)
    b_attn = np.asarray(b_attn, np.float32)
    w_proj = np.asarray(w_proj, np.float32)
    b_proj = np.asarray(b_proj, np.float32)

    nc1 = _get("l1", _build_l1)
    res1 = run_bass_kernel_spmd(nc1, _prep_l1_inputs(x, w_attn, b_attn),
                                list(range(NCORES))).results

    QT_all = np.concatenate([res1[c]["qkt"][:C] for c in range(NCORES)], axis=1)
    KT_all = np.concatenate([res1[c]["qkt"][C:] for c in range(NCORES)], axis=1)
    V_all = np.concatenate([res1[c]["vo"] for c in range(NCORES)], axis=0)
    Vp = np.ones((T, H, HD + 1), np.float32)
    Vp[:, :, :HD] = np.asarray(V_all, np.float32).reshape(T, H, HD)
    bp1 = b_proj + b_attn[2 * C:] @ w_proj

    nc2 = _get("l2", _build_l2)
    res2 = run_bass_kernel_spmd(nc2, _prep_l2_inputs(QT_all, KT_all, Vp,
                                                     w_proj, bp1),
                                list(range(NCORES))).results

    out = np.empty((T, C), np.float32)
    for c in range(NCORES):
        for s, b in enumerate(BLOCKS[c]):
            out[128 * b:128 * (b + 1)] = res2[c]["out"][128 * s:128 * (s + 1)]
    return out[None]
